# revision 1
# baseline (speedup 1.0000x reference)
"""Causal multi-head attention (B=1, S=4096, D=1024, H=16, hd=64), fp32,
sharded over 8 trn2 NeuronCores: 2 heads per core (tensor parallel),
row-sharded Wo, host-side partial-sum reduce.

kernel(**inputs) takes full unsharded inputs, returns full output.
"""

import sys

import numpy as np

for _p in ("/opt/trn_rl_repo", "/root/.axon_site/_ro/trn_rl_repo"):
    if _p not in sys.path:
        sys.path.insert(0, _p)

import concourse.bass as bass  # noqa: E402
import concourse.tile as tile  # noqa: E402
from concourse import bacc, mybir  # noqa: E402
from concourse.bass_utils import run_bass_kernel_spmd  # noqa: E402
from concourse.masks import make_identity  # noqa: E402

F32 = mybir.dt.float32
F32R = mybir.dt.float32r
BF16 = mybir.dt.bfloat16

N_CORES = 8
FPC = 128  # features per core (2 heads x 64)
HD = 64


def build_nc(S=4096, D=1024, repeat=1, no_out_dma=False, hoist_x=False,
             no_exp=False, no_dve_extras=False, act_small=False,
             dve_small=False):
    """Build the per-core bass program (same program for all 8 cores; each
    core receives its own weight slices)."""
    NDC = D // 128          # d-chunks (contraction tiles)
    NKT = S // 128          # k tiles
    NQC = S // 512          # q chunks
    HLEN = min(2048, S)     # token half length
    NHALF = S // HLEN
    NJ = HLEN // 512        # 512-col chunks per half

    nc = bacc.Bacc("TRN2", target_bir_lowering=False, debug=False,
                   num_devices=N_CORES)

    xT = nc.dram_tensor("xT", [D, S], F32, kind="ExternalInput")
    wq = nc.dram_tensor("wq", [128, D], F32, kind="ExternalInput")
    wk = nc.dram_tensor("wk", [128, D], F32, kind="ExternalInput")
    wv = nc.dram_tensor("wv", [128, D], F32, kind="ExternalInput")
    wo = nc.dram_tensor("wo", [FPC, D], F32, kind="ExternalInput")
    tri = nc.dram_tensor("tri", [128, 128], F32, kind="ExternalInput")
    out = nc.dram_tensor("out", [S, D], F32, kind="ExternalOutput")

    with tile.TileContext(nc) as tc:
        with tc.tile_pool(name="const", bufs=1) as const, \
             tc.tile_pool(name="persist", bufs=1) as persist, \
             tc.tile_pool(name="xpool", bufs=16) as xpool, \
             tc.tile_pool(name="vTpool", bufs=2) as vTpool, \
             tc.tile_pool(name="eppool", bufs=6) as eppool, \
             tc.tile_pool(name="smalls", bufs=4) as smalls, \
             tc.tile_pool(name="outsb", bufs=5) as outsb, \
             tc.tile_pool(name="qkvps", bufs=2, space="PSUM") as qkvps, \
             tc.tile_pool(name="scps", bufs=2, space="PSUM") as scps, \
             tc.tile_pool(name="ctxps", bufs=1, space="PSUM") as ctxps:
            # constants
            wq_sb = const.tile([128, D], F32R, tag="wq")
            wk_sb = const.tile([128, D], F32R, tag="wk")
            wv_sb = const.tile([128, D], F32R, tag="wv")
            wo_sb = const.tile([FPC, D], F32R, tag="wo")
            tri_sb = const.tile([128, 128], F32, tag="tri")
            ident = const.tile([128, 128], F32, tag="ident")
            nc.scalar.dma_start(out=wq_sb[:], in_=wq[:].bitcast(F32R))
            nc.scalar.dma_start(out=wk_sb[:], in_=wk[:].bitcast(F32R))
            nc.scalar.dma_start(out=wv_sb[:], in_=wv[:].bitcast(F32R))
            nc.scalar.dma_start(out=wo_sb[:], in_=wo[:].bitcast(F32R))
            nc.scalar.dma_start(out=tri_sb[:], in_=tri[:])
            make_identity(nc, ident[:])

            # persistent intermediates
            qT = persist.tile([128, S], BF16, tag="qT")    # [feat, tok]
            kT = persist.tile([128, S], BF16, tag="kT")
            v_sb = persist.tile([128, NKT * 130], F32R, tag="v_sb")
            ctxT = persist.tile([128, S], F32R, tag="ctxT")

            # ones columns of v_sb (cols kt*130+64+65h) — written once
            for _kt in range(NKT):
                for _h in range(2):
                    nc.vector.memset(
                        v_sb[:, _kt * 130 + 65 * _h + 64:
                                _kt * 130 + 65 * _h + 65].bitcast(F32), 1.0)

            for _rep in range(repeat):

                def emit_half(half, after_j=None):
                    vT_half = vTpool.tile([128, HLEN], F32, tag="vT",
                                          name="vT_half")
                    for j in range(NJ):
                        xc = []
                        for c8 in range(NDC):
                            xt = xpool.tile([128, 512], F32R, tag="x",
                                            name="x")
                            col0 = half * HLEN + j * 512
                            nc.sync.dma_start(
                                out=xt[:],
                                in_=xT[c8 * 128:(c8 + 1) * 128,
                                       col0:col0 + 512].bitcast(F32R))
                            xc.append(xt)
                        for (w_sb, kind) in ((wq_sb, "q"), (wk_sb, "k"),
                                             (wv_sb, "v")):
                            ps = qkvps.tile([128, 512], F32, tag="qkv",
                                            name="ps")
                            for c8 in range(NDC):
                                nc.tensor.matmul(
                                    ps[:], w_sb[:, c8 * 128:(c8 + 1) * 128],
                                    xc[c8][:],
                                    start=(c8 == 0), stop=(c8 == NDC - 1))
                            col = half * HLEN + j * 512
                            if kind == "q":
                                nc.vector.tensor_copy(qT[:, col:col + 512],
                                                      ps[:])
                            elif kind == "k":
                                nc.vector.tensor_copy(kT[:, col:col + 512],
                                                      ps[:])
                            else:
                                nc.vector.tensor_copy(
                                    vT_half[:, j * 512:(j + 1) * 512], ps[:])
                        # transpose this j-block of V into token-major v_sb
                        for t in range(4 * j, 4 * j + 4):
                            kt = half * (HLEN // 128) + t
                            pst = qkvps.tile([128, 512], F32, tag="qkv",
                                             name="pst")
                            nc.tensor.transpose(
                                pst[:, 0:128],
                                vT_half[:, t * 128:(t + 1) * 128], ident[:])
                            dst = v_sb[:, kt * 130:kt * 130 + 130] \
                                .rearrange("p (g u) -> p g u", g=2,
                                           u=65)[:, :, 0:64]
                            src = pst[:, 0:128].rearrange(
                                "p (g u) -> p g u", g=2, u=64)
                            nc.vector.tensor_copy(dst, src)
                        if after_j is not None:
                            after_j(half * HLEN + (j + 1) * 512)

                state = {"prev": None}

                def emit_norm(qc_, ctx_):
                    for h in range(2):
                        rrow = smalls.tile([1, 512], F32, tag="rrow",
                                           name="rrow")
                        nc.vector.reciprocal(rrow[:], ctx_[h][64:65, :])
                        rb = smalls.tile([64, 512], F32, tag="rb", name="rb")
                        nc.gpsimd.partition_broadcast(rb[:], rrow[:])
                        nc.vector.tensor_mul(
                            ctxT[64 * h:64 * h + 64,
                                 qc_ * 512:(qc_ + 1) * 512],
                            ctx_[h][0:64, :].bitcast(F32R),
                            rb[:].bitcast(F32R))

                def emit_outproj(qc_, ts_=(0, 1, 2, 3)):
                    for t in ts_:
                        qt = qc_ * 4 + t
                        ot = outsb.tile([128, D], F32, tag="ot", name="ot")
                        for g in range(D // 512):
                            po = qkvps.tile([128, 512], F32, tag="qkv",
                                            name="po")
                            nc.tensor.matmul(
                                po[:], ctxT[:, qt * 128:(qt + 1) * 128],
                                wo_sb[:, g * 512:(g + 1) * 512],
                                start=True, stop=True)
                            nc.vector.tensor_copy(
                                ot[:, g * 512:g * 512 + (64 if dve_small
                                                         else 512)],
                                po[:, 0:(64 if dve_small else 512)])
                        if not no_out_dma:
                            nc.scalar.dma_start(
                                out=out[qt * 128:(qt + 1) * 128, :], in_=ot[:])

                def emit_chunk(qc):
                    kmax = 4 * qc + 4
                    ctx = []
                    for h in range(2):
                        cx = ctxps.tile([65, 512], F32, tag=f"ctx{h}",
                                        name=f"ctx{h}")
                        ctx.append(cx)
                    pend = []

                    def emit_ctx(args, ctx=ctx, kmax=kmax):
                        kt_, ep_, s0_ = args
                        for h in range(2):
                            nc.tensor.matmul(
                                ctx[h][:, s0_:512],
                                v_sb[:, kt_ * 130 + 65 * h:
                                     kt_ * 130 + 65 * h + 65],
                                ep_[:, h * 512 + s0_:(h + 1) * 512],
                                start=(kt_ == 0), stop=(kt_ == kmax - 1),
                                skip_group_check=True)

                    for kt in range(kmax):
                        s0 = max(0, kt * 128 - qc * 512)
                        sc = scps.tile([128, 1024], F32, tag="sc", name="sc")
                        for h in range(2):
                            nc.tensor.matmul(
                                sc[:, h * 512 + s0:h * 512 + 512],
                                kT[64 * h:64 * h + 64,
                                   kt * 128:(kt + 1) * 128],
                                qT[64 * h:64 * h + 64,
                                   qc * 512 + s0:(qc + 1) * 512],
                                start=True, stop=True)
                        ep = eppool.tile([128, 1024], F32R, tag="ep",
                                         name="ep")
                        sc_v = sc[:].rearrange("p (g u) -> p g u",
                                               g=2)[:, :, s0:512]
                        ep_v = ep[:].rearrange("p (g u) -> p g u",
                                               g=2)[:, :, s0:512]
                        if act_small:
                            sc_v = sc[:].rearrange(
                                "p (g u) -> p g u", g=2)[:, :, s0:s0 + 64]
                            ep_v = ep[:].rearrange(
                                "p (g u) -> p g u", g=2)[:, :, s0:s0 + 64]
                        if not no_exp:
                            nc.scalar.activation(
                                ep_v, sc_v, mybir.ActivationFunctionType.Exp,
                                scale=0.125)
                        if kt >= 4 * qc and not no_dve_extras:
                            j0 = kt * 128 - qc * 512
                            for h in range(2):
                                blk = ep[:, h * 512 + j0:h * 512 + j0 + 128]
                                nc.gpsimd.affine_select(
                                    out=blk, in_=blk,
                                    compare_op=mybir.AluOpType.is_ge,
                                    fill=0.0, base=0,
                                    pattern=[[1, 128]],
                                    channel_multiplier=-1)
                        pend.append((kt, ep, s0))
                        if len(pend) > 2:
                            emit_ctx(pend.pop(0))
                        if kt == 1 and state["prev"] is not None:
                            emit_norm(*state["prev"])
                        if state["prev"] is not None and 3 <= kt < 7 \
                                and kmax > 7:
                            emit_outproj(state["prev"][0], (kt - 3,))
                        elif kt == 3 and kmax <= 7 \
                                and state["prev"] is not None:
                            emit_outproj(state["prev"][0])
                    while pend:
                        emit_ctx(pend.pop(0))
                    state["prev"] = (qc, ctx)

                prog = {"chunks": 0}

                def after_j(tokens_done):
                    while (prog["chunks"] + 1) * 512 <= tokens_done \
                            and prog["chunks"] < NQC:
                        emit_chunk(prog["chunks"])
                        prog["chunks"] += 1

                for half in range(NHALF):
                    emit_half(half, after_j=after_j)
                while prog["chunks"] < NQC:
                    emit_chunk(prog["chunks"])
                    prog["chunks"] += 1
                emit_norm(*state["prev"])
                emit_outproj(state["prev"][0])

    nc.compile()
    return nc


_NC_CACHE = {}


def _get_nc(S, D):
    key = (S, D)
    if key not in _NC_CACHE:
        _NC_CACHE[key] = build_nc(S, D)
    return _NC_CACHE[key]


def make_in_maps(x, Wq, Wk, Wv, Wo, S, D):
    xT = np.ascontiguousarray(np.asarray(x, dtype=np.float32).reshape(S, D).T)
    tri = (np.arange(128)[:, None] <= np.arange(128)[None, :]) \
        .astype(np.float32)
    NDC = D // 128
    in_maps = []
    for c in range(N_CORES):
        sl = slice(c * FPC, (c + 1) * FPC)

        def prep(W):
            # [D, 128] -> [128(part within chunk), NDC*128]
            return np.ascontiguousarray(
                np.asarray(W[:, sl], dtype=np.float32)
                .reshape(NDC, 128, FPC).transpose(1, 0, 2).reshape(128, D))

        in_maps.append({
            "xT": xT,
            "wq": prep(Wq),
            "wk": prep(Wk),
            "wv": prep(Wv),
            "wo": np.ascontiguousarray(np.asarray(Wo[sl, :], dtype=np.float32)),
            "tri": tri,
        })
    return in_maps


def kernel(x, Wq, Wk, Wv, Wo, bo):
    x = np.asarray(x, dtype=np.float32)
    B, S, D = x.shape
    nc = _get_nc(S, D)
    in_maps = make_in_maps(x, Wq, Wk, Wv, Wo, S, D)
    res = run_bass_kernel_spmd(nc, in_maps, core_ids=list(range(N_CORES)))
    acc = np.zeros((S, D), dtype=np.float32)
    for c in range(N_CORES):
        acc += res.results[c]["out"]
    acc += np.asarray(bo, dtype=np.float32)[None, :]
    return acc.reshape(B, S, D)

